# revision 10
# baseline (speedup 1.0000x reference)
"""Bass/Trainium2 kernel for nn_MAC_30554397344312 (gnn_message_passing).

Reference computation (B=256 rollout groups, n=64 agents, D=256):
    comm = h @ W_act.T + b_act                      # (B*n, D)
    agg[b,j] = sum_i mask[i,j] * comm[b,i] / (n-1)  # mask = ones - eye
    x   = agg @ W_sum.T + b_sum
    out = relu(x @ W_head.T + b_head)

Everything before the relu is linear, so fold on host:
    Wc = W_head @ W_sum @ W_act          (256x256)
    bc = b_head + b_sum @ W_head.T + b_act @ (W_head @ W_sum).T
    out[b,j] = relu( (A @ H_b)[j] @ Wc.T + bc ),  A = (ones-eye)/(n-1)

On device (per core, 2048 rows = 16 token tiles of 128):
    stage 1 (GPSIMD): cast h chunks f32 -> fp16 (keeps DVE off this path).
    stage 2 (PE): Y.T tiles [d, tok] via matmul(lhsT=H_tile[128tok,128d],
                  rhs=blockdiag(A,A)) - aggregation and transpose fused.
    stage 3 (DVE): evict Y.T PSUM bank to SBUF (one [128,512] copy/chunk).
    stage 4 (PE): out[tok, d_out] = Y.T.T @ Wc.T accumulated over 2 k-chunks.
    stage 5 (ACT/DVE alternating): relu + scale + PSUM->SBUF evict
                  (one [128,512] op per chunk).
    stage 6: per-chunk DMA store.

Pipelined at 2-tile (256 KiB) granularity (1-tile head/tail chunks for fast
fill/drain); input DMA, PE, GPSIMD, DVE/ACT and output DMA all overlap.
Loads strictly precede stores on each HWDGE ring to avoid head-of-line
blocking; chunks alternate rings.

Sharding: data-parallel over the B axis, 8 cores x 2048 rows.
"""

from contextlib import ExitStack

import numpy as np

import concourse.bacc as bacc
import concourse.bass as bass
import concourse.tile as tile
from concourse import mybir
from concourse.bass_utils import run_bass_kernel_spmd

N_AGENTS = 64
B = 256
D = 256
N_CORES = 8
ROWS = B * N_AGENTS            # 16384
ROWS_PER_CORE = ROWS // N_CORES  # 2048
P = 128
N_TILES = ROWS_PER_CORE // P   # 16 token tiles per core
# chunk plan: (tile_start, n_tiles); 1-tile head and tail, 2-tile body
CHUNKS = [(0, 1)] + [(1 + 2 * i, 2) for i in range(7)] + [(15, 1)]
W_SCALE = 16.0  # fp16 weight prescale (power of 2; inverted exactly in relu)

_cache = {}


def _build(has_bias: bool, f16: bool = True):
    f32 = mybir.dt.float32
    mdt = mybir.dt.float16 if f16 else mybir.dt.float32
    inv_scale = 1.0 / W_SCALE if f16 else 1.0
    nc = bacc.Bacc("TRN2", target_bir_lowering=False, debug=False,
                   num_devices=N_CORES)

    h = nc.dram_tensor("h", [ROWS_PER_CORE, D], f32, kind="ExternalInput")
    wcT = nc.dram_tensor("wcT", [D, D], mdt, kind="ExternalInput")
    ablk = nc.dram_tensor("ablk", [P, P], mdt, kind="ExternalInput")
    if has_bias:
        bc = nc.dram_tensor("bc", [1, D], f32, kind="ExternalInput")
    out = nc.dram_tensor("out", [ROWS_PER_CORE, D], f32, kind="ExternalOutput")

    h_ap = h[:, :].rearrange("(n p) d -> p n d", p=P)      # [128, 16, 256]
    out_ap = out[:, :].rearrange("(n p) d -> p n d", p=P)  # [128, 16, 256]
    w_ap = wcT[:, :].rearrange("(k p) d -> p k d", p=P)    # [128, 2, 256]

    NCH = len(CHUNKS)

    with tile.TileContext(nc) as tc:
        with ExitStack() as ctx:
            const = ctx.enter_context(tc.tile_pool(name="const", bufs=1))
            aggps = ctx.enter_context(
                tc.tile_pool(name="aggps", bufs=3, space="PSUM"))
            outps = ctx.enter_context(
                tc.tile_pool(name="outps", bufs=3, space="PSUM"))
            warmps = ctx.enter_context(
                tc.tile_pool(name="warmps", bufs=1, space="PSUM"))

            a_t = const.tile([P, P], mdt, tag="a", name="a_t")
            w_t = const.tile([P, 2, D], mdt, tag="w", name="w_t")
            if has_bias:
                bc_t = const.tile([P, D], f32, tag="bc", name="bc_t")

            # ---- PE warm-up: dependency-free matmuls on scratch data so the
            # HAM clock gate releases (1.2 -> 2.4 GHz) while input DMA runs;
            # the real matmul stream then starts at full clock.
            ws_t = const.tile([P, 4 * P], mdt, tag="ws", name="ws_t")
            nc.gpsimd.memset(ws_t[:], 0.0)
            wp_t = warmps.tile([P, 4 * P], f32, tag="wp", name="wp_t")
            for _ in range(7):
                nc.tensor.matmul(wp_t[:], ws_t[:, :P], ws_t[:],
                                 start=True, stop=True)

            # ---- input DMA issue: loads first on each ring (alternating),
            # weights right behind the first scalar-ring chunk.
            nc.sync.dma_start(out=a_t[:], in_=ablk[:, :])
            traw = []
            for c, (t0, nt) in enumerate(CHUNKS):
                t = const.tile([P, nt, D], f32, tag=f"hr{c}", name=f"hr_{c}")
                eng = nc.sync if c % 2 == 0 else nc.scalar
                eng.dma_start(out=t[:], in_=h_ap[:, t0:t0 + nt, :])
                traw.append(t)
                if c == 1:
                    nc.scalar.dma_start(out=w_t[:], in_=w_ap)
                    if has_bias:
                        bc_bcast = bass.AP(
                            tensor=bc, offset=0, ap=[[0, P], [1, D]])
                        nc.gpsimd.dma_start(out=bc_t[:], in_=bc_bcast)

            # fp16 views of h chunks (DVE cast)
            hc = [const.tile([P, nt, D], mdt, tag=f"hc{c}", name=f"hc_{c}")
                  for c, (t0, nt) in enumerate(CHUNKS)] if f16 else traw

            # Y.T in SBUF: [128 d, 2 k-chunks, 2048 tok] single tile
            yt = const.tile([P, 2, ROWS_PER_CORE], mdt, tag="yt", name="yt")
            och = [const.tile([P, nt, D], f32, tag=f"oc{c}", name=f"oc_{c}")
                   for c, (t0, nt) in enumerate(CHUNKS)]

            def cast(c):
                if f16:
                    nc.vector.tensor_copy(out=hc[c][:], in_=traw[c][:])

            def agg(c):
                t0, nt = CHUNKS[c]
                # one PSUM bank per chunk, k-major columns [k, s, 128]
                ps = aggps.tile([P, 2, nt * P], f32, tag="aggps",
                                name="agg_ps")
                for s in range(nt):
                    for k in range(2):
                        lhsT = hc[c][:, s, k * P:(k + 1) * P]
                        nc.tensor.matmul(
                            ps[:, k, s * P:(s + 1) * P], lhsT, a_t[:],
                            start=True, stop=True)
                # single DVE evict for the whole chunk (both k halves)
                nc.vector.tensor_copy(
                    out=yt[:, :, t0 * P:(t0 + nt) * P], in_=ps[:])

            def main(c):
                t0, nt = CHUNKS[c]
                po = outps.tile([P, nt, D], f32, tag="outps", name="po")
                for s in range(nt):
                    m = t0 + s
                    for k in range(2):
                        nc.tensor.matmul(
                            po[:, s, :], yt[:, k, m * P:(m + 1) * P],
                            w_t[:, k, :], start=(k == 0), stop=(k == 1))
                dst = och[c][:]
                if has_bias:
                    for s in range(nt):
                        nc.vector.tensor_scalar(
                            out=och[c][:, s, :], in0=po[:, s, :],
                            scalar1=inv_scale, scalar2=None,
                            op0=mybir.AluOpType.mult)
                        nc.vector.tensor_tensor(
                            out=och[c][:, s, :], in0=och[c][:, s, :],
                            in1=bc_t[:], op=mybir.AluOpType.add)
                        nc.scalar.activation(
                            out=och[c][:, s, :], in_=och[c][:, s, :],
                            func=mybir.ActivationFunctionType.Relu)
                elif c % 2 == 0:
                    nc.scalar.activation(
                        out=dst, in_=po[:],
                        func=mybir.ActivationFunctionType.Relu,
                        scale=inv_scale)
                else:
                    nc.vector.tensor_scalar(
                        out=dst, in0=po[:], scalar1=inv_scale,
                        scalar2=0.0, op0=mybir.AluOpType.mult,
                        op1=mybir.AluOpType.max)
                # stores avoid the scalar ring (busy with ACT relus): even
                # chunks ride the SWDGE ring, odd the sync ring (idle by
                # then); the final stores take HWDGE rings (lower completion
                # latency on the critical tail).
                if c == NCH - 1:
                    seng = nc.sync
                elif c == NCH - 2:
                    seng = nc.scalar
                else:
                    seng = nc.gpsimd if c % 2 == 0 else nc.sync
                seng.dma_start(out=out_ap[:, t0:t0 + nt, :], in_=och[c][:])

            # one-chunk lookahead keeps PE busy while DVE evicts Y.T
            cast(0)
            agg(0)
            cast(1)
            agg(1)
            for c in range(NCH - 2):
                main(c)
                cast(c + 2)
                agg(c + 2)
            main(NCH - 2)
            main(NCH - 1)
    nc.finalize()
    return nc


def _fold(W_act, b_act, W_sum, b_sum, W_head, b_head, f16=True):
    Wa = W_act.astype(np.float64)
    Ws = W_sum.astype(np.float64)
    Wh = W_head.astype(np.float64)
    Wc = Wh @ Ws @ Wa
    bc = (b_head.astype(np.float64)
          + b_sum.astype(np.float64) @ Wh.T
          + b_act.astype(np.float64) @ (Wh @ Ws).T)
    A = np.ones((N_AGENTS, N_AGENTS)) - np.eye(N_AGENTS)
    if f16:
        # mask stays exact 0/1 in fp16; 1/63 and the fp16-subnormal
        # prescale fold into the weights, inverted via the relu scale.
        WcT = (Wc.T / (N_AGENTS - 1) * W_SCALE).astype(np.float16)
        wdt = np.float16
    else:
        A = A / (N_AGENTS - 1)
        WcT = Wc.T.astype(np.float32)
        wdt = np.float32
    Ablk = np.zeros((P, P))
    Ablk[:N_AGENTS, :N_AGENTS] = A
    Ablk[N_AGENTS:, N_AGENTS:] = A
    return (np.ascontiguousarray(WcT), bc.astype(np.float32),
            Ablk.astype(wdt))


def kernel(hidden_state, W_act, b_act, W_sum, b_sum, W_head, b_head,
           _trace=False, _tmpdir=None):
    import os
    f16 = os.environ.get("KERNEL_F32", "0") != "1"
    h = np.ascontiguousarray(np.asarray(hidden_state, dtype=np.float32))
    WcT, bc, Ablk = _fold(np.asarray(W_act), np.asarray(b_act),
                          np.asarray(W_sum), np.asarray(b_sum),
                          np.asarray(W_head), np.asarray(b_head), f16=f16)
    has_bias = bool(np.any(bc))
    if (has_bias, f16) not in _cache:
        _cache[(has_bias, f16)] = _build(has_bias, f16=f16)
    nc = _cache[(has_bias, f16)]

    in_maps = []
    for c in range(N_CORES):
        m = {"h": h[c * ROWS_PER_CORE:(c + 1) * ROWS_PER_CORE],
             "wcT": WcT, "ablk": Ablk}
        if has_bias:
            m["bc"] = bc.reshape(1, D)
        in_maps.append(m)

    res = run_bass_kernel_spmd(
        nc, in_maps, core_ids=list(range(N_CORES)),
        trace=_trace, tmpdir=_tmpdir)
    out = np.concatenate([res.results[c]["out"] for c in range(N_CORES)],
                         axis=0)
    if _trace:
        return out, res
    return out


# revision 11
# speedup vs baseline: 1.0576x; 1.0576x over previous
"""Bass/Trainium2 kernel for nn_MAC_30554397344312 (gnn_message_passing).

Reference computation (B=256 rollout groups, n=64 agents, D=256):
    comm = h @ W_act.T + b_act                      # (B*n, D)
    agg[b,j] = sum_i mask[i,j] * comm[b,i] / (n-1)  # mask = ones - eye
    x   = agg @ W_sum.T + b_sum
    out = relu(x @ W_head.T + b_head)

Everything before the relu is linear, so fold on host:
    Wc = W_head @ W_sum @ W_act          (256x256)
    bc = b_head + b_sum @ W_head.T + b_act @ (W_head @ W_sum).T
    out[b,j] = relu( (A @ H_b)[j] @ Wc.T + bc ),  A = (ones-eye)/(n-1)

On device (per core, 2048 rows = 16 token tiles of 128):
    loads  (SWDGE): h chunks DMA'd with inline f32 -> fp16 cast, so no
                    on-chip cast stage at all; loads own the gpsimd ring,
                    stores own the two HWDGE rings (R/W streams overlap).
    stage 1 (PE): Y.T tiles [d, tok] via matmul(lhsT=H_tile[128tok,128d],
                  rhs=blockdiag(A,A)) - aggregation and transpose fused.
    stage 2 (DVE): evict Y.T PSUM bank to SBUF (one [128,512] copy/chunk).
    stage 3 (PE): out[tok, d_out] = Y.T.T @ Wc.T accumulated over 2 k-chunks.
    stage 4 (ACT/DVE alternating): relu + scale + PSUM->SBUF evict.
    stage 5: per-chunk DMA store, alternating HWDGE rings.

A short burst of dependency-free warm-up matmuls runs right after the
preamble barrier so the PE HAM clock gate releases (1.2 -> 2.4 GHz)
before the real matmul stream arrives.

Sharding: data-parallel over the B axis, 8 cores x 2048 rows.
"""

from contextlib import ExitStack

import numpy as np

import concourse.bacc as bacc
import concourse.bass as bass
import concourse.tile as tile
from concourse import mybir
from concourse.bass_utils import run_bass_kernel_spmd

N_AGENTS = 64
B = 256
D = 256
N_CORES = 8
ROWS = B * N_AGENTS            # 16384
ROWS_PER_CORE = ROWS // N_CORES  # 2048
P = 128
N_TILES = ROWS_PER_CORE // P   # 16 token tiles per core
# chunk plan: (tile_start, n_tiles); 1-tile head pair for fast pipeline fill
CHUNKS = [(0, 1), (1, 1)] + [(2 + 2 * i, 2) for i in range(7)]
W_SCALE = 16.0  # fp16 weight prescale (power of 2; inverted exactly in relu)

_cache = {}


def _build(has_bias: bool, f16: bool = True):
    f32 = mybir.dt.float32
    mdt = mybir.dt.float16 if f16 else mybir.dt.float32
    inv_scale = 1.0 / W_SCALE if f16 else 1.0
    nc = bacc.Bacc("TRN2", target_bir_lowering=False, debug=False,
                   num_devices=N_CORES)

    h = nc.dram_tensor("h", [ROWS_PER_CORE, D], f32, kind="ExternalInput")
    wcT = nc.dram_tensor("wcT", [D, D], mdt, kind="ExternalInput")
    ablk = nc.dram_tensor("ablk", [P, P], mdt, kind="ExternalInput")
    if has_bias:
        bc = nc.dram_tensor("bc", [1, D], f32, kind="ExternalInput")
    out = nc.dram_tensor("out", [ROWS_PER_CORE, D], f32, kind="ExternalOutput")

    h_ap = h[:, :].rearrange("(n p) d -> p n d", p=P)      # [128, 16, 256]
    out_ap = out[:, :].rearrange("(n p) d -> p n d", p=P)  # [128, 16, 256]
    w_ap = wcT[:, :].rearrange("(k p) d -> p k d", p=P)    # [128, 2, 256]

    NCH = len(CHUNKS)

    with tile.TileContext(nc) as tc:
        with ExitStack() as ctx:
            const = ctx.enter_context(tc.tile_pool(name="const", bufs=1))
            aggps = ctx.enter_context(
                tc.tile_pool(name="aggps", bufs=3, space="PSUM"))
            outps = ctx.enter_context(
                tc.tile_pool(name="outps", bufs=3, space="PSUM"))
            warmps = ctx.enter_context(
                tc.tile_pool(name="warmps", bufs=1, space="PSUM"))

            a_t = const.tile([P, P], mdt, tag="a", name="a_t")
            w_t = const.tile([P, 2, D], mdt, tag="w", name="w_t")
            if has_bias:
                bc_t = const.tile([P, D], f32, tag="bc", name="bc_t")

            # ---- PE warm-up: dependency-free matmuls on scratch data so the
            # HAM clock gate releases while input DMA runs.
            ws_t = const.tile([P, 4 * P], mdt, tag="ws", name="ws_t")
            nc.vector.memset(ws_t[:], 0.0)
            wp_t = warmps.tile([P, 4 * P], f32, tag="wp", name="wp_t")
            for _ in range(3):
                nc.tensor.matmul(wp_t[:], ws_t[:, :P], ws_t[:],
                                 start=True, stop=True)

            # ---- weights on the (otherwise store-only) HWDGE rings
            nc.sync.dma_start(out=a_t[:], in_=ablk[:, :])
            nc.scalar.dma_start(out=w_t[:], in_=w_ap)
            if has_bias:
                bc_bcast = bass.AP(tensor=bc, offset=0, ap=[[0, P], [1, D]])
                nc.gpsimd.dma_start(out=bc_t[:], in_=bc_bcast)

            # ---- input: SWDGE loads with inline f32 -> fp16 cast
            hc = []
            for c, (t0, nt) in enumerate(CHUNKS):
                t = const.tile([P, nt, D], mdt, tag=f"hc{c}", name=f"hc_{c}")
                if f16:
                    nc.gpsimd.dma_start(out=t[:], in_=h_ap[:, t0:t0 + nt, :])
                else:
                    nc.sync.dma_start(out=t[:], in_=h_ap[:, t0:t0 + nt, :])
                hc.append(t)

            # Y.T in SBUF: [128 d, 2 k-chunks, 2048 tok] single tile
            yt = const.tile([P, 2, ROWS_PER_CORE], mdt, tag="yt", name="yt")
            och = [const.tile([P, nt, D], f32, tag=f"oc{c}", name=f"oc_{c}")
                   for c, (t0, nt) in enumerate(CHUNKS)]

            def agg(c):
                t0, nt = CHUNKS[c]
                # one PSUM bank per chunk, k-major columns [k, s, 128]
                ps = aggps.tile([P, 2, nt * P], f32, tag="aggps",
                                name="agg_ps")
                for s in range(nt):
                    for k in range(2):
                        lhsT = hc[c][:, s, k * P:(k + 1) * P]
                        nc.tensor.matmul(
                            ps[:, k, s * P:(s + 1) * P], lhsT, a_t[:],
                            start=True, stop=True)
                # single DVE evict for the whole chunk (both k halves)
                nc.vector.tensor_copy(
                    out=yt[:, :, t0 * P:(t0 + nt) * P], in_=ps[:])

            def main(c):
                t0, nt = CHUNKS[c]
                po = outps.tile([P, nt, D], f32, tag="outps", name="po")
                for s in range(nt):
                    m = t0 + s
                    for k in range(2):
                        nc.tensor.matmul(
                            po[:, s, :], yt[:, k, m * P:(m + 1) * P],
                            w_t[:, k, :], start=(k == 0), stop=(k == 1))
                dst = och[c][:]
                if has_bias:
                    for s in range(nt):
                        nc.vector.tensor_scalar(
                            out=och[c][:, s, :], in0=po[:, s, :],
                            scalar1=inv_scale, scalar2=None,
                            op0=mybir.AluOpType.mult)
                        nc.vector.tensor_tensor(
                            out=och[c][:, s, :], in0=och[c][:, s, :],
                            in1=bc_t[:], op=mybir.AluOpType.add)
                        nc.scalar.activation(
                            out=och[c][:, s, :], in_=och[c][:, s, :],
                            func=mybir.ActivationFunctionType.Relu)
                elif c % 2 == 0:
                    nc.scalar.activation(
                        out=dst, in_=po[:],
                        func=mybir.ActivationFunctionType.Relu,
                        scale=inv_scale)
                else:
                    nc.vector.tensor_scalar(
                        out=dst, in0=po[:], scalar1=inv_scale,
                        scalar2=0.0, op0=mybir.AluOpType.mult,
                        op1=mybir.AluOpType.max)
                # stores alternate the two HWDGE rings (loads are on SWDGE)
                (nc.sync if c % 2 == 0 else nc.scalar).dma_start(
                    out=out_ap[:, t0:t0 + nt, :], in_=och[c][:])

            # one-chunk lookahead keeps PE busy while DVE evicts Y.T
            agg(0)
            agg(1)
            for c in range(NCH - 2):
                main(c)
                agg(c + 2)
            main(NCH - 2)
            main(NCH - 1)
    nc.finalize()
    return nc


def _fold(W_act, b_act, W_sum, b_sum, W_head, b_head, f16=True):
    Wa = W_act.astype(np.float64)
    Ws = W_sum.astype(np.float64)
    Wh = W_head.astype(np.float64)
    Wc = Wh @ Ws @ Wa
    bc = (b_head.astype(np.float64)
          + b_sum.astype(np.float64) @ Wh.T
          + b_act.astype(np.float64) @ (Wh @ Ws).T)
    A = np.ones((N_AGENTS, N_AGENTS)) - np.eye(N_AGENTS)
    if f16:
        # mask stays exact 0/1 in fp16; 1/63 and the fp16-subnormal
        # prescale fold into the weights, inverted via the relu scale.
        WcT = (Wc.T / (N_AGENTS - 1) * W_SCALE).astype(np.float16)
        wdt = np.float16
    else:
        A = A / (N_AGENTS - 1)
        WcT = Wc.T.astype(np.float32)
        wdt = np.float32
    Ablk = np.zeros((P, P))
    Ablk[:N_AGENTS, :N_AGENTS] = A
    Ablk[N_AGENTS:, N_AGENTS:] = A
    return (np.ascontiguousarray(WcT), bc.astype(np.float32),
            Ablk.astype(wdt))


def kernel(hidden_state, W_act, b_act, W_sum, b_sum, W_head, b_head,
           _trace=False, _tmpdir=None):
    import os
    f16 = os.environ.get("KERNEL_F32", "0") != "1"
    h = np.ascontiguousarray(np.asarray(hidden_state, dtype=np.float32))
    WcT, bc, Ablk = _fold(np.asarray(W_act), np.asarray(b_act),
                          np.asarray(W_sum), np.asarray(b_sum),
                          np.asarray(W_head), np.asarray(b_head), f16=f16)
    has_bias = bool(np.any(bc))
    if (has_bias, f16) not in _cache:
        _cache[(has_bias, f16)] = _build(has_bias, f16=f16)
    nc = _cache[(has_bias, f16)]

    in_maps = []
    for c in range(N_CORES):
        m = {"h": h[c * ROWS_PER_CORE:(c + 1) * ROWS_PER_CORE],
             "wcT": WcT, "ablk": Ablk}
        if has_bias:
            m["bc"] = bc.reshape(1, D)
        in_maps.append(m)

    res = run_bass_kernel_spmd(
        nc, in_maps, core_ids=list(range(N_CORES)),
        trace=_trace, tmpdir=_tmpdir)
    out = np.concatenate([res.results[c]["out"] for c in range(N_CORES)],
                         axis=0)
    if _trace:
        return out, res
    return out


# revision 17
# speedup vs baseline: 1.0583x; 1.0006x over previous
"""Bass/Trainium2 kernel for nn_MAC_30554397344312 (gnn_message_passing).

Reference computation (B=256 rollout groups, n=64 agents, D=256):
    comm = h @ W_act.T + b_act                      # (B*n, D)
    agg[b,j] = sum_i mask[i,j] * comm[b,i] / (n-1)  # mask = ones - eye
    x   = agg @ W_sum.T + b_sum
    out = relu(x @ W_head.T + b_head)

Everything before the relu is linear, so fold on host:
    Wc = W_head @ W_sum @ W_act          (256x256)
    bc = b_head + b_sum @ W_head.T + b_act @ (W_head @ W_sum).T
    out[b,j] = relu( (A @ H_b)[j] @ Wc.T + bc ),  A = (ones-eye)/(n-1)

On device (per core, 2048 rows = 16 token tiles of 128):
    loads  (SWDGE): h chunks DMA'd with inline f32 -> fp16 cast, so no
                    on-chip cast stage at all; loads own the gpsimd ring,
                    stores own the two HWDGE rings (R/W streams overlap).
    stage 1 (PE): Y.T tiles [d, tok] via matmul(lhsT=H_tile[128tok,128d],
                  rhs=blockdiag(A,A)) - aggregation and transpose fused.
    stage 2 (DVE): evict Y.T PSUM bank to SBUF (one [128,512] copy/chunk).
    stage 3 (PE): out[tok, d_out] = Y.T.T @ Wc.T accumulated over 2 k-chunks.
    stage 4 (ACT/DVE alternating): relu + scale + PSUM->SBUF evict.
    stage 5: per-chunk DMA store, alternating HWDGE rings.

A short burst of dependency-free warm-up matmuls runs right after the
preamble barrier so the PE HAM clock gate releases (1.2 -> 2.4 GHz)
before the real matmul stream arrives.

Sharding: data-parallel over the B axis, 8 cores x 2048 rows.
"""

from contextlib import ExitStack

import numpy as np

import concourse.bacc as bacc
import concourse.bass as bass
import concourse.tile as tile
from concourse import mybir
from concourse.bass_utils import run_bass_kernel_spmd

N_AGENTS = 64
B = 256
D = 256
N_CORES = 8
ROWS = B * N_AGENTS            # 16384
ROWS_PER_CORE = ROWS // N_CORES  # 2048
P = 128
N_TILES = ROWS_PER_CORE // P   # 16 token tiles per core
# chunk plan: (tile_start, n_tiles); small head chunk for fast pipeline
# fill, bigger body chunks to amortize the ~0.85us SWDGE per-DMA floor
CHUNKS = [(0, 1), (1, 2), (3, 2), (5, 3), (8, 3), (11, 3), (14, 2)]
W_SCALE = 16.0  # fp16 weight prescale (power of 2; inverted exactly in relu)

_cache = {}


def _build(has_bias: bool, f16: bool = True):
    f32 = mybir.dt.float32
    mdt = mybir.dt.float16 if f16 else mybir.dt.float32
    inv_scale = 1.0 / W_SCALE if f16 else 1.0
    nc = bacc.Bacc("TRN2", target_bir_lowering=False, debug=False,
                   num_devices=N_CORES)

    h = nc.dram_tensor("h", [ROWS_PER_CORE, D], f32, kind="ExternalInput")
    wcT = nc.dram_tensor("wcT", [D, D], mdt, kind="ExternalInput")
    ablk = nc.dram_tensor("ablk", [P, P], mdt, kind="ExternalInput")
    if has_bias:
        bc = nc.dram_tensor("bc", [1, D], f32, kind="ExternalInput")
    out = nc.dram_tensor("out", [ROWS_PER_CORE, D], f32, kind="ExternalOutput")

    h_ap = h[:, :].rearrange("(n p) d -> p n d", p=P)      # [128, 16, 256]
    out_ap = out[:, :].rearrange("(n p) d -> p n d", p=P)  # [128, 16, 256]
    w_ap = wcT[:, :].rearrange("(k p) d -> p k d", p=P)    # [128, 2, 256]

    NCH = len(CHUNKS)

    with tile.TileContext(nc) as tc:
        with ExitStack() as ctx:
            const = ctx.enter_context(tc.tile_pool(name="const", bufs=1))
            aggps = ctx.enter_context(
                tc.tile_pool(name="aggps", bufs=2, space="PSUM"))
            outps = ctx.enter_context(
                tc.tile_pool(name="outps", bufs=2, space="PSUM"))

            a_t = const.tile([P, P], mdt, tag="a", name="a_t")
            w_t = const.tile([P, 2, D], mdt, tag="w", name="w_t")
            if has_bias:
                bc_t = const.tile([P, D], f32, tag="bc", name="bc_t")

            # ---- PE warm-up: dependency-free matmuls on scratch data so the
            # HAM clock gate releases (1.2 -> 2.4 GHz) before real work; the
            # burst bridges the input-DMA latency window (~3.4us busy needed).
            ws_t = const.tile([P, 4 * P], mdt, tag="ws", name="ws_t")
            nc.vector.memset(ws_t[:], 0.0)
            wp_t = outps.tile([P, 3, D], f32, tag="outps", name="wp_t")
            for _ in range(6):
                nc.tensor.matmul(wp_t[:, 0:2, :], ws_t[:, :P],
                                 ws_t[:], start=True, stop=True)

            # ---- weights on the (otherwise store-only) HWDGE rings
            nc.sync.dma_start(out=a_t[:], in_=ablk[:, :])
            nc.scalar.dma_start(out=w_t[:], in_=w_ap)
            if has_bias:
                bc_bcast = bass.AP(tensor=bc, offset=0, ap=[[0, P], [1, D]])
                nc.gpsimd.dma_start(out=bc_t[:], in_=bc_bcast)

            # ---- input: SWDGE loads with inline f32 -> fp16 cast
            hc = []
            for c, (t0, nt) in enumerate(CHUNKS):
                t = const.tile([P, nt, D], mdt, tag=f"hc{c}", name=f"hc_{c}")
                if f16:
                    nc.gpsimd.dma_start(out=t[:], in_=h_ap[:, t0:t0 + nt, :])
                else:
                    nc.sync.dma_start(out=t[:], in_=h_ap[:, t0:t0 + nt, :])
                hc.append(t)

            # Y.T in SBUF: [128 d, 2 k-chunks, 2048 tok] single tile
            yt = const.tile([P, 2, ROWS_PER_CORE], mdt, tag="yt", name="yt")
            och = [const.tile([P, nt, D], f32, tag=f"oc{c}", name=f"oc_{c}")
                   for c, (t0, nt) in enumerate(CHUNKS)]

            def agg(c):
                t0, nt = CHUNKS[c]
                # one PSUM region per chunk, k-major columns [k, s, 128]
                ps = aggps.tile([P, 2, 3 * P], f32, tag="aggps",
                                name="agg_ps")
                ps = ps[:, :, :nt * P]
                for s in range(nt):
                    for k in range(2):
                        lhsT = hc[c][:, s, k * P:(k + 1) * P]
                        nc.tensor.matmul(
                            ps[:, k, s * P:(s + 1) * P], lhsT, a_t[:],
                            start=True, stop=True)
                # single DVE evict for the whole chunk (both k halves)
                nc.vector.tensor_copy(
                    out=yt[:, :, t0 * P:(t0 + nt) * P], in_=ps[:])

            def main(c):
                t0, nt = CHUNKS[c]
                po = outps.tile([P, 3, D], f32, tag="outps", name="po")
                po = po[:, :nt, :]
                for s in range(nt):
                    m = t0 + s
                    for k in range(2):
                        nc.tensor.matmul(
                            po[:, s, :], yt[:, k, m * P:(m + 1) * P],
                            w_t[:, k, :], start=(k == 0), stop=(k == 1))
                dst = och[c][:]
                if has_bias:
                    for s in range(nt):
                        nc.vector.tensor_scalar(
                            out=och[c][:, s, :], in0=po[:, s, :],
                            scalar1=inv_scale, scalar2=None,
                            op0=mybir.AluOpType.mult)
                        nc.vector.tensor_tensor(
                            out=och[c][:, s, :], in0=och[c][:, s, :],
                            in1=bc_t[:], op=mybir.AluOpType.add)
                        nc.scalar.activation(
                            out=och[c][:, s, :], in_=och[c][:, s, :],
                            func=mybir.ActivationFunctionType.Relu)
                elif c % 2 == 0:
                    nc.scalar.activation(
                        out=dst, in_=po[:],
                        func=mybir.ActivationFunctionType.Relu,
                        scale=inv_scale)
                else:
                    nc.vector.tensor_scalar(
                        out=dst, in0=po[:], scalar1=inv_scale,
                        scalar2=0.0, op0=mybir.AluOpType.mult,
                        op1=mybir.AluOpType.max)
                # stores alternate the two HWDGE rings (loads are on SWDGE)
                (nc.sync if c % 2 == 0 else nc.scalar).dma_start(
                    out=out_ap[:, t0:t0 + nt, :], in_=och[c][:])

            # one-chunk lookahead keeps PE busy while DVE evicts Y.T
            agg(0)
            agg(1)
            for c in range(NCH - 2):
                main(c)
                agg(c + 2)
            main(NCH - 2)
            main(NCH - 1)
    nc.finalize()
    return nc


def _fold(W_act, b_act, W_sum, b_sum, W_head, b_head, f16=True):
    Wa = W_act.astype(np.float64)
    Ws = W_sum.astype(np.float64)
    Wh = W_head.astype(np.float64)
    Wc = Wh @ Ws @ Wa
    bc = (b_head.astype(np.float64)
          + b_sum.astype(np.float64) @ Wh.T
          + b_act.astype(np.float64) @ (Wh @ Ws).T)
    A = np.ones((N_AGENTS, N_AGENTS)) - np.eye(N_AGENTS)
    if f16:
        # mask stays exact 0/1 in fp16; 1/63 and the fp16-subnormal
        # prescale fold into the weights, inverted via the relu scale.
        WcT = (Wc.T / (N_AGENTS - 1) * W_SCALE).astype(np.float16)
        wdt = np.float16
    else:
        A = A / (N_AGENTS - 1)
        WcT = Wc.T.astype(np.float32)
        wdt = np.float32
    Ablk = np.zeros((P, P))
    Ablk[:N_AGENTS, :N_AGENTS] = A
    Ablk[N_AGENTS:, N_AGENTS:] = A
    return (np.ascontiguousarray(WcT), bc.astype(np.float32),
            Ablk.astype(wdt))


def kernel(hidden_state, W_act, b_act, W_sum, b_sum, W_head, b_head,
           _trace=False, _tmpdir=None):
    import os
    f16 = os.environ.get("KERNEL_F32", "0") != "1"
    h = np.ascontiguousarray(np.asarray(hidden_state, dtype=np.float32))
    WcT, bc, Ablk = _fold(np.asarray(W_act), np.asarray(b_act),
                          np.asarray(W_sum), np.asarray(b_sum),
                          np.asarray(W_head), np.asarray(b_head), f16=f16)
    has_bias = bool(np.any(bc))
    if (has_bias, f16) not in _cache:
        _cache[(has_bias, f16)] = _build(has_bias, f16=f16)
    nc = _cache[(has_bias, f16)]

    in_maps = []
    for c in range(N_CORES):
        m = {"h": h[c * ROWS_PER_CORE:(c + 1) * ROWS_PER_CORE],
             "wcT": WcT, "ablk": Ablk}
        if has_bias:
            m["bc"] = bc.reshape(1, D)
        in_maps.append(m)

    res = run_bass_kernel_spmd(
        nc, in_maps, core_ids=list(range(N_CORES)),
        trace=_trace, tmpdir=_tmpdir)
    out = np.concatenate([res.results[c]["out"] for c in range(N_CORES)],
                         axis=0)
    if _trace:
        return out, res
    return out


# revision 21
# speedup vs baseline: 1.0826x; 1.0230x over previous
"""Bass/Trainium2 kernel for nn_MAC_30554397344312 (gnn_message_passing).

Reference computation (B=256 rollout groups, n=64 agents, D=256):
    comm = h @ W_act.T + b_act                      # (B*n, D)
    agg[b,j] = sum_i mask[i,j] * comm[b,i] / (n-1)  # mask = ones - eye
    x   = agg @ W_sum.T + b_sum
    out = relu(x @ W_head.T + b_head)

Everything before the relu is linear, so fold on host:
    Wc = W_head @ W_sum @ W_act          (256x256)
    bc = b_head + b_sum @ W_head.T + b_act @ (W_head @ W_sum).T
    out[b,j] = relu( (A @ H_b)[j] @ Wc.T + bc ),  A = (ones-eye)/(n-1)

On device (per core, 2048 rows = 16 token tiles of 128):
    loads  (SWDGE): h chunks DMA'd with inline f32 -> fp16 cast, so no
                    on-chip cast stage at all; loads own the gpsimd ring,
                    stores own the two HWDGE rings (R/W streams overlap).
    stage 1 (PE): Y.T tiles [d, tok] via matmul(lhsT=H_tile[128tok,128d],
                  rhs=blockdiag(A,A)) - aggregation and transpose fused.
    stage 2 (DVE): evict Y.T PSUM bank to SBUF (one [128,512] copy/chunk).
    stage 3 (PE): out[tok, d_out] = Y.T.T @ Wc.T accumulated over 2 k-chunks.
    stage 4 (ACT/DVE alternating): relu + scale + PSUM->SBUF evict.
    stage 5: per-chunk DMA store, alternating HWDGE rings.

A short burst of dependency-free warm-up matmuls runs right after the
preamble barrier so the PE HAM clock gate releases (1.2 -> 2.4 GHz)
before the real matmul stream arrives.

Sharding: data-parallel over the B axis, 8 cores x 2048 rows.
"""

from contextlib import ExitStack

import numpy as np

import concourse.bacc as bacc
import concourse.bass as bass
import concourse.tile as tile
from concourse import mybir
from concourse.bass_utils import run_bass_kernel_spmd

N_AGENTS = 64
B = 256
D = 256
N_CORES = 8
ROWS = B * N_AGENTS            # 16384
ROWS_PER_CORE = ROWS // N_CORES  # 2048
P = 128
N_TILES = ROWS_PER_CORE // P   # 16 token tiles per core
# chunk plan: (tile_start, n_tiles); bigger chunks amortize the ~1.1us
# SWDGE per-DMA floor so input streams at the HBM-read rate
CHUNKS = [(0, 2), (2, 4), (6, 4), (10, 3), (13, 3)]
MAXT = 4                       # max tiles per chunk (PSUM tile sizing)
W_SCALE = 16.0  # fp16 weight prescale (power of 2; inverted exactly in relu)

_cache = {}


def _build(has_bias: bool, f16: bool = True):
    f32 = mybir.dt.float32
    mdt = mybir.dt.float16 if f16 else mybir.dt.float32
    inv_scale = 1.0 / W_SCALE if f16 else 1.0
    nc = bacc.Bacc("TRN2", target_bir_lowering=False, debug=False,
                   num_devices=N_CORES)

    h = nc.dram_tensor("h", [ROWS_PER_CORE, D], f32, kind="ExternalInput")
    wcT = nc.dram_tensor("wcT", [D, D], mdt, kind="ExternalInput")
    ablk = nc.dram_tensor("ablk", [P, P], mdt, kind="ExternalInput")
    if has_bias:
        bc = nc.dram_tensor("bc", [1, D], f32, kind="ExternalInput")
    out = nc.dram_tensor("out", [ROWS_PER_CORE, D], f32, kind="ExternalOutput")

    h_ap = h[:, :].rearrange("(n p) d -> p n d", p=P)      # [128, 16, 256]
    out_ap = out[:, :].rearrange("(n p) d -> p n d", p=P)  # [128, 16, 256]
    w_ap = wcT[:, :].rearrange("(k p) d -> p k d", p=P)    # [128, 2, 256]

    NCH = len(CHUNKS)

    with tile.TileContext(nc) as tc:
        with ExitStack() as ctx:
            const = ctx.enter_context(tc.tile_pool(name="const", bufs=1))
            aggps = ctx.enter_context(
                tc.tile_pool(name="aggps", bufs=2, space="PSUM"))
            outps = ctx.enter_context(
                tc.tile_pool(name="outps", bufs=2, space="PSUM"))

            a_t = const.tile([P, P], mdt, tag="a", name="a_t")
            w_t = const.tile([P, 2, D], mdt, tag="w", name="w_t")
            if has_bias:
                bc_t = const.tile([P, D], f32, tag="bc", name="bc_t")

            # ---- PE warm-up: dependency-free matmuls on scratch data so the
            # HAM clock gate releases (1.2 -> 2.4 GHz) before real work; the
            # burst bridges the input-DMA latency window (~3.4us busy needed).
            ws_t = const.tile([P, 4 * P], mdt, tag="ws", name="ws_t")
            nc.vector.memset(ws_t[:], 0.0)
            wp_t = outps.tile([P, MAXT, D], f32, tag="outps", name="wp_t")
            for _ in range(9):
                nc.tensor.matmul(wp_t[:, 0:2, :], ws_t[:, :P],
                                 ws_t[:], start=True, stop=True)

            # ---- weights on the (otherwise store-only) HWDGE rings
            nc.sync.dma_start(out=a_t[:], in_=ablk[:, :])
            nc.scalar.dma_start(out=w_t[:], in_=w_ap)
            if has_bias:
                bc_bcast = bass.AP(tensor=bc, offset=0, ap=[[0, P], [1, D]])
                nc.gpsimd.dma_start(out=bc_t[:], in_=bc_bcast)

            # ---- input: SWDGE loads with inline f32 -> fp16 cast
            hc = []
            for c, (t0, nt) in enumerate(CHUNKS):
                t = const.tile([P, nt, D], mdt, tag=f"hc{c}", name=f"hc_{c}")
                if f16:
                    nc.gpsimd.dma_start(out=t[:], in_=h_ap[:, t0:t0 + nt, :])
                else:
                    nc.sync.dma_start(out=t[:], in_=h_ap[:, t0:t0 + nt, :])
                hc.append(t)

            # Y.T in SBUF: [128 d, 2 k-chunks, 2048 tok] single tile
            yt = const.tile([P, 2, ROWS_PER_CORE], mdt, tag="yt", name="yt")
            och = [const.tile([P, nt, D], f32, tag=f"oc{c}", name=f"oc_{c}")
                   for c, (t0, nt) in enumerate(CHUNKS)]

            def agg(c):
                t0, nt = CHUNKS[c]
                # one PSUM region per chunk, k-major columns [k, s, 128]
                ps = aggps.tile([P, 2, MAXT * P], f32, tag="aggps",
                                name="agg_ps")
                ps = ps[:, :, :nt * P]
                for s in range(nt):
                    for k in range(2):
                        lhsT = hc[c][:, s, k * P:(k + 1) * P]
                        nc.tensor.matmul(
                            ps[:, k, s * P:(s + 1) * P], lhsT, a_t[:],
                            start=True, stop=True)
                # single DVE evict for the whole chunk (both k halves)
                nc.vector.tensor_copy(
                    out=yt[:, :, t0 * P:(t0 + nt) * P], in_=ps[:])

            def main(c):
                t0, nt = CHUNKS[c]
                po = outps.tile([P, MAXT, D], f32, tag="outps", name="po")
                po = po[:, :nt, :]
                for s in range(nt):
                    m = t0 + s
                    for k in range(2):
                        nc.tensor.matmul(
                            po[:, s, :], yt[:, k, m * P:(m + 1) * P],
                            w_t[:, k, :], start=(k == 0), stop=(k == 1))
                dst = och[c][:]
                if has_bias:
                    for s in range(nt):
                        nc.vector.tensor_scalar(
                            out=och[c][:, s, :], in0=po[:, s, :],
                            scalar1=inv_scale, scalar2=None,
                            op0=mybir.AluOpType.mult)
                        nc.vector.tensor_tensor(
                            out=och[c][:, s, :], in0=och[c][:, s, :],
                            in1=bc_t[:], op=mybir.AluOpType.add)
                        nc.scalar.activation(
                            out=och[c][:, s, :], in_=och[c][:, s, :],
                            func=mybir.ActivationFunctionType.Relu)
                elif c % 2 == 0:
                    nc.scalar.activation(
                        out=dst, in_=po[:],
                        func=mybir.ActivationFunctionType.Relu,
                        scale=inv_scale)
                else:
                    nc.vector.tensor_scalar(
                        out=dst, in0=po[:], scalar1=inv_scale,
                        scalar2=0.0, op0=mybir.AluOpType.mult,
                        op1=mybir.AluOpType.max)
                # stores alternate the two HWDGE rings (loads are on SWDGE)
                (nc.sync if c % 2 == 0 else nc.scalar).dma_start(
                    out=out_ap[:, t0:t0 + nt, :], in_=och[c][:])

            # one-chunk lookahead keeps PE busy while DVE evicts Y.T
            agg(0)
            agg(1)
            for c in range(NCH - 2):
                main(c)
                agg(c + 2)
            main(NCH - 2)
            main(NCH - 1)
    nc.finalize()
    return nc


def _fold(W_act, b_act, W_sum, b_sum, W_head, b_head, f16=True):
    Wa = W_act.astype(np.float64)
    Ws = W_sum.astype(np.float64)
    Wh = W_head.astype(np.float64)
    Wc = Wh @ Ws @ Wa
    bc = (b_head.astype(np.float64)
          + b_sum.astype(np.float64) @ Wh.T
          + b_act.astype(np.float64) @ (Wh @ Ws).T)
    A = np.ones((N_AGENTS, N_AGENTS)) - np.eye(N_AGENTS)
    if f16:
        # mask stays exact 0/1 in fp16; 1/63 and the fp16-subnormal
        # prescale fold into the weights, inverted via the relu scale.
        WcT = (Wc.T / (N_AGENTS - 1) * W_SCALE).astype(np.float16)
        wdt = np.float16
    else:
        A = A / (N_AGENTS - 1)
        WcT = Wc.T.astype(np.float32)
        wdt = np.float32
    Ablk = np.zeros((P, P))
    Ablk[:N_AGENTS, :N_AGENTS] = A
    Ablk[N_AGENTS:, N_AGENTS:] = A
    return (np.ascontiguousarray(WcT), bc.astype(np.float32),
            Ablk.astype(wdt))


def kernel(hidden_state, W_act, b_act, W_sum, b_sum, W_head, b_head,
           _trace=False, _tmpdir=None):
    import os
    f16 = os.environ.get("KERNEL_F32", "0") != "1"
    h = np.ascontiguousarray(np.asarray(hidden_state, dtype=np.float32))
    WcT, bc, Ablk = _fold(np.asarray(W_act), np.asarray(b_act),
                          np.asarray(W_sum), np.asarray(b_sum),
                          np.asarray(W_head), np.asarray(b_head), f16=f16)
    has_bias = bool(np.any(bc))
    if (has_bias, f16) not in _cache:
        _cache[(has_bias, f16)] = _build(has_bias, f16=f16)
    nc = _cache[(has_bias, f16)]

    in_maps = []
    for c in range(N_CORES):
        m = {"h": h[c * ROWS_PER_CORE:(c + 1) * ROWS_PER_CORE],
             "wcT": WcT, "ablk": Ablk}
        if has_bias:
            m["bc"] = bc.reshape(1, D)
        in_maps.append(m)

    res = run_bass_kernel_spmd(
        nc, in_maps, core_ids=list(range(N_CORES)),
        trace=_trace, tmpdir=_tmpdir)
    out = np.concatenate([res.results[c]["out"] for c in range(N_CORES)],
                         axis=0)
    if _trace:
        return out, res
    return out
